# revision 10
# baseline (speedup 1.0000x reference)
"""GAT (2-layer, 4-head) distributed Bass kernel for Trainium2, 8 NeuronCores.

v3 design:
  - fp16 packed featel table: rows of 512 B ([feat(128 f16) | el(4 f16) | pad]).
  - Table is split into two halves by local node row (tiles 0-24 -> table A,
    tiles 25-48 -> table B). Each half is AllGathered separately so the
    second half's collective overlaps other work, and each half has
    < 32768 rows so int16 gather indices need no rebasing tricks.
  - NO er dma_gather: er stays in SBUF per-core; the per-edge expansion
    er_e = er[dstrow_e] is a tiny PE matmul with a host-PREBUILT transposed
    one-hot OT; aggregation agg = O^T @ [featw | s] uses the prebuilt O.
    O/OT are compile-time constants streamed from DRAM as fp16 slabs.
  - dma_gather calls round-robin over 4 SWDGE queues (parallel Q7 prep).
  - Layer-2 node matmuls (pre-phase) are fused into the layer-1 epilogue,
    so the layer-2 AllGathers fire while layer-1 edge processing runs.
  - Single-pass softmax: alpha = exp(e)/sum(exp(e)); logits are O(1).
"""

import numpy as np

# ---- problem constants (hardcoded; kernel.py must be self-contained) ----
N = 50000
E = 800000
P = 8
IN = 128
HID = 32
H = 4
F = H * HID          # 128, same for both layers
OUTD = 32
NEG = 0.2
TILE = 128

NLOC = N // P        # 6250
T = (NLOC + TILE - 1) // TILE          # 49
NLOCP = T * TILE     # 6272
TA = 31              # tiles in table half A (8*TA*128 <= 32767)
TB = T - TA          # 24
RA = TA * TILE       # 3200 rows per core, half A
RB = TB * TILE       # 3072 rows per core, half B

ROWE = 256           # featel table row elems (f16) -> 512 B
NQ = 4               # SWDGE queues


def _wrap16(idx):
    """[n] index list -> [128, n//16] int16, wrapped in 16 partitions and
    replicated across the 8 Q7 cores (dma_gather layout)."""
    a = np.asarray(idx).reshape(-1, 16).T
    return np.tile(a, (8, 1)).astype(np.int16)


# ----------------------------------------------------------------------------
# Host-side preprocessing
# ----------------------------------------------------------------------------

def prep_edges(src, dst, n=N, p=P):
    """Group edges by (dst owner, dst tile, src table-half), pad each
    (core,tile,half) to common chunk counts KA_t/KB_t, and emit per-core
    index arrays plus prebuilt one-hot slabs.

    Returns (KAs, KBs, per_core): per_core[c] has
      gA   int16 [128, 8*sumKA]   wrapped gather idxs into table A
      gB   int16 [128, 8*sumKB]   wrapped gather idxs into table B
      Otab  f16  [128, sumK*128]  per-chunk one-hot O[e, r] (pad rows: 0)
      OTtab f16  [128, sumK*128]  per-chunk transposed one-hot OT[r, e]
    """
    nloc = n // p
    t_tiles = (nloc + TILE - 1) // TILE
    assert t_tiles == T

    owner = dst // nloc
    loc = dst - owner * nloc
    tl = loc // TILE
    row = loc - tl * TILE

    sowner = src // nloc
    sloc = src - sowner * nloc
    hb = (sloc >= RA).astype(np.int64)            # 0 = table A, 1 = table B
    pgid = np.where(hb == 0, sowner * RA + sloc, sowner * RB + (sloc - RA))

    order = np.lexsort((hb, tl, owner))
    owner_s = owner[order]
    tl_s = tl[order]
    hb_s = hb[order]
    row_s = row[order].astype(np.int64)
    pgid_s = pgid[order].astype(np.int64)

    counts = np.zeros((p, t_tiles, 2), dtype=np.int64)
    np.add.at(counts, (owner_s, tl_s, hb_s), 1)
    KAs = (-(-counts[:, :, 0] // TILE)).max(axis=0)
    KBs = (-(-counts[:, :, 1] // TILE)).max(axis=0)
    KAs = np.maximum(KAs, (KAs + KBs) == 0)       # ensure >=1 chunk per tile
    Ks = KAs + KBs
    off = np.concatenate([[0], np.cumsum(Ks)]).astype(int)
    offA = np.concatenate([[0], np.cumsum(KAs)]).astype(int)
    offB = np.concatenate([[0], np.cumsum(KBs)]).astype(int)
    sumK, sumKA, sumKB = int(off[-1]), int(offA[-1]), int(offB[-1])

    grp = (owner_s * t_tiles + tl_s) * 2 + hb_s
    gcnt = np.bincount(grp, minlength=p * t_tiles * 2)
    gstart = np.concatenate([[0], np.cumsum(gcnt)])
    within = np.arange(len(src)) - gstart[grp]
    k = within // TILE
    prt = within - k * TILE
    # chunk column in the full per-tile layout (A chunks first, then B)
    col = off[tl_s] + np.where(hb_s == 0, k, KAs[tl_s] + k)

    per_core = []
    for c in range(p):
        m = owner_s == c
        dstrow = np.full((TILE, sumK), -1, dtype=np.int64)
        dstrow[prt[m], col[m]] = row_s[m]
        gfull = np.zeros((TILE, sumK), dtype=np.int64)
        gfull[prt[m], col[m]] = pgid_s[m]

        # prebuilt one-hots: O[e, r] = (dstrow[e]==r), pad slots (-1) all-zero
        eye = np.concatenate([np.eye(TILE, dtype=np.float16),
                              np.zeros((1, TILE), np.float16)])  # row -1 -> 0
        Otab = np.empty((TILE, sumK * TILE), dtype=np.float16)
        OTtab = np.empty((TILE, sumK * TILE), dtype=np.float16)
        for q in range(sumK):
            Oq = eye[dstrow[:, q]]                # [128e, 128r]
            Otab[:, q * TILE:(q + 1) * TILE] = Oq
            OTtab[:, q * TILE:(q + 1) * TILE] = Oq.T

        # flatten chunk cols into wrapped idx streams
        gA = np.zeros((TILE, 8 * sumKA), dtype=np.int16)
        gB = np.zeros((TILE, 8 * sumKB), dtype=np.int16)
        for t in range(t_tiles):
            ka, kb = int(KAs[t]), int(KBs[t])
            o, oa, ob = off[t], offA[t], offB[t]
            if ka:
                ia = gfull[:, o:o + ka].T.reshape(-1)          # i = k*128+p
                gA[:, 8 * oa:8 * (oa + ka)] = _wrap16(ia)
            if kb:
                ib = gfull[:, o + ka:o + ka + kb].T.reshape(-1)
                gB[:, 8 * ob:8 * (ob + kb)] = _wrap16(ib)
        per_core.append(dict(gA=gA, gB=gB, Otab=Otab, OTtab=OTtab))
    return [int(x) for x in KAs], [int(x) for x in KBs], per_core


def prep_weights(W, al, ar):
    """[W | W@al per head | W@ar per head] -> [in, F+2H] float16."""
    Wr = W.reshape(W.shape[0], H, -1)
    wal = np.einsum('ihd,hd->ih', Wr, al)
    war = np.einsum('ihd,hd->ih', Wr, ar)
    return np.concatenate([W, wal, war], axis=1).astype(np.float16)


def prep_node_inputs(x, b1, n=N, p=P):
    """Per-core xT ([IN, NLOCP] f16, lhsT layout) and xb ([128, T*IN] f32,
    tile-row-major residual layout, bias prefolded)."""
    nloc = n // p
    t_tiles = (nloc + TILE - 1) // TILE
    nlocp = t_tiles * TILE
    outs = []
    for c in range(p):
        xl = np.zeros((nlocp, x.shape[1]), dtype=np.float32)
        xl[:nloc] = x[c * nloc:(c + 1) * nloc]
        xT = np.ascontiguousarray(xl.T).astype(np.float16)
        xb = (xl + b1[None, :]).reshape(t_tiles, TILE, -1).transpose(1, 0, 2)
        xb = np.ascontiguousarray(xb.reshape(TILE, -1))
        outs.append((xT, xb))
    return outs


# ----------------------------------------------------------------------------
# Bass IR builder
# ----------------------------------------------------------------------------

def build_gat(KAs, KBs, n=N, p=P, in_dim=IN):
    import concourse.bass as bass
    import concourse.bacc as bacc
    import concourse.mybir as mybir
    import concourse.tile as tile

    f32 = mybir.dt.float32
    f16 = mybir.dt.float16
    i16 = mybir.dt.int16
    AF = mybir.ActivationFunctionType
    ALU = mybir.AluOpType

    t_tiles = T
    KAs = list(KAs)
    KBs = list(KBs)
    Ks = [a + b for a, b in zip(KAs, KBs)]
    off = np.concatenate([[0], np.cumsum(Ks)]).astype(int)
    offA = np.concatenate([[0], np.cumsum(KAs)]).astype(int)
    offB = np.concatenate([[0], np.cumsum(KBs)]).astype(int)
    sumK, sumKA, sumKB = int(off[-1]), int(offA[-1]), int(offB[-1])
    Kmax = max(Ks)
    rg = [list(range(p))]

    nc = bacc.Bacc("TRN2", target_bir_lowering=False, num_swdge_queues=NQ)

    # ---- I/O ----
    xT_in = nc.dram_tensor("xT", [in_dim, NLOCP], f16, kind="ExternalInput")
    xb_in = nc.dram_tensor("xb", [TILE, t_tiles * in_dim], f32, kind="ExternalInput")
    W1_in = nc.dram_tensor("Wcat1", [in_dim, F + 2 * H], f16, kind="ExternalInput")
    W2_in = nc.dram_tensor("Wcat2", [F, F + 2 * H], f16, kind="ExternalInput")
    b2r_in = nc.dram_tensor("b2r", [TILE, F], f32, kind="ExternalInput")
    ident_in = nc.dram_tensor("ident", [TILE, TILE], f32, kind="ExternalInput")
    ones1_in = nc.dram_tensor("ones1", [1, TILE], f16, kind="ExternalInput")
    negc2_in = nc.dram_tensor("negc2", [1, F + 2 * H], f16, kind="ExternalInput")
    gA_in = nc.dram_tensor("gA", [TILE, 8 * sumKA], i16, kind="ExternalInput")
    gB_in = nc.dram_tensor("gB", [TILE, max(8 * sumKB, 16)], i16, kind="ExternalInput")
    Otab_in = nc.dram_tensor("Otab", [TILE, sumK * TILE], f16, kind="ExternalInput")
    OTtab_in = nc.dram_tensor("OTtab", [TILE, sumK * TILE], f16, kind="ExternalInput")
    out_ext = nc.dram_tensor("out", [NLOCP, OUTD], f32, kind="ExternalOutput")

    # ---- internal DRAM (per layer, per table half) ----
    warm_loc = nc.dram_tensor("warm_loc", [1, ROWE], f16)
    warm_full = nc.dram_tensor("warm_full", [p, ROWE], f16, addr_space="Shared")
    felA_loc = [nc.dram_tensor(f"felA_loc{i}", [RA, ROWE], f16) for i in (1, 2)]
    felB_loc = [nc.dram_tensor(f"felB_loc{i}", [RB, ROWE], f16) for i in (1, 2)]
    felA_full = [nc.dram_tensor(f"felA_full{i}", [p * RA, ROWE], f16,
                                addr_space="Shared") for i in (1, 2)]
    felB_full = [nc.dram_tensor(f"felB_full{i}", [p * RB, ROWE], f16,
                                addr_space="Shared") for i in (1, 2)]

    qrr = [0]  # SWDGE queue round-robin

    with tile.TileContext(nc) as tc:
        with tc.tile_pool(name="cst", bufs=1) as cst, \
             tc.tile_pool(name="big", bufs=1) as big, \
             tc.tile_pool(name="fe", bufs=5) as fep, \
             tc.tile_pool(name="osl", bufs=4) as osl, \
             tc.tile_pool(name="xbp", bufs=3) as xbp, \
             tc.tile_pool(name="wk", bufs=6) as wk, \
             tc.tile_pool(name="ep", bufs=3) as ep, \
             tc.tile_pool(name="ps", bufs=1, space="PSUM") as ps:

            xT = cst.sbuf_tile_from(xT_in.ap())
            Wc1 = cst.sbuf_tile_from(W1_in.ap())
            Wc2 = cst.sbuf_tile_from(W2_in.ap())
            b2r = cst.sbuf_tile_from(b2r_in.ap())
            ident = cst.sbuf_tile_from(ident_in.ap())
            ones1 = cst.sbuf_tile_from(ones1_in.ap())
            negc2 = cst.sbuf_tile_from(negc2_in.ap())
            gA = cst.sbuf_tile_from(gA_in.ap())
            gB = cst.sbuf_tile_from(gB_in.ap())

            h_sb = big.tile([TILE, t_tiles * F], f32)
            hT_sb = big.tile([TILE, t_tiles * TILE], f16)
            er_sb = [big.tile([TILE, t_tiles * H], f16, name=f"er{i}")
                     for i in (0, 1)]

            def pre_tile(lhsT_sb, Wc, layer, nt, corr=False):
                """one tile of node-level matmul -> featel_loc (f16) + er_sb.
                corr=True accumulates -colsum(Wc) (the h+1 offset fix)."""
                sl = slice(nt * TILE, (nt + 1) * TILE)
                pf = ps.tile([TILE, F + 2 * H], f32, tag="pf", bufs=2,
                             name=f"pf{layer}_{nt}")
                nc.tensor.matmul(pf[:, :], lhsT=lhsT_sb[:, sl],
                                 rhs=Wc[:, :], start=True, stop=not corr)
                if corr:
                    nc.tensor.matmul(pf[:, :], lhsT=ones1[:, :],
                                     rhs=negc2[:, :], start=False, stop=True)
                fel = ep.tile([TILE, ROWE], f16, tag="fel", name=f"fel{layer}_{nt}")
                nc.scalar.activation(fel[:, 0:F + H], pf[:, 0:F + H], AF.Copy)
                nc.scalar.activation(er_sb[layer][:, nt * H:(nt + 1) * H],
                                     pf[:, F + H:F + 2 * H], AF.Copy)
                if nt < TA:
                    nc.sync.dma_start(
                        felA_loc[layer][nt * TILE:(nt + 1) * TILE, :], fel[:, :])
                else:
                    nc.sync.dma_start(
                        felB_loc[layer][(nt - TA) * TILE:(nt - TA + 1) * TILE, :],
                        fel[:, :])

            def allgather(layer, half):
                loc, full = ((felA_loc, felA_full) if half == 0
                             else (felB_loc, felB_full))
                nc.gpsimd.collective_compute(
                    "AllGather", mybir.AluOpType.bypass, replica_groups=rg,
                    ins=[loc[layer].ap().opt()], outs=[full[layer].ap().opt()])

            def gather_call(fe_slice, src_ap, idx_slice, nrows):
                q = qrr[0] % NQ
                qrr[0] += 1
                nc.gpsimd.dma_gather(
                    fe_slice, src_ap, idx_slice, nrows, nrows, ROWE,
                    single_packet=False, queue_num=q)

            def edge_phase(layer):
                """per-dst-tile gather + SDDMM + softmax-weighted aggregation."""
                for t in range(t_tiles):
                    ka, kb = KAs[t], KBs[t]
                    kt = ka + kb
                    o0, oa, ob = int(off[t]), int(offA[t]), int(offB[t])
                    fe = fep.tile([TILE, kt, ROWE], f16, tag="fe",
                                  padded_shape=[TILE, Kmax, ROWE], name=f"fe{layer}_{t}")
                    for g0 in range(0, ka, 8):
                        gk = min(8, ka - g0)
                        gather_call(fe[:, g0:g0 + gk, :], felA_full[layer].ap(),
                                    gA[:, 8 * (oa + g0):8 * (oa + g0 + gk)],
                                    gk * TILE)
                    for g0 in range(0, kb, 8):
                        gk = min(8, kb - g0)
                        gather_call(fe[:, ka + g0:ka + g0 + gk, :],
                                    felB_full[layer].ap(),
                                    gB[:, 8 * (ob + g0):8 * (ob + g0 + gk)],
                                    gk * TILE)
                    # O / OT slabs (prebuilt one-hots, contiguous stream)
                    osb = osl.tile([TILE, kt, TILE], f16, tag="osb",
                                   padded_shape=[TILE, Kmax, TILE], name=f"os{layer}_{t}")
                    nc.sync.dma_start(osb[:, :, :],
                                      Otab_in[:, o0 * TILE:(o0 + kt) * TILE])
                    otsb = osl.tile([TILE, kt, TILE], f16, tag="otsb",
                                    padded_shape=[TILE, Kmax, TILE], name=f"ot{layer}_{t}")
                    nc.sync.dma_start(otsb[:, :, :],
                                      OTtab_in[:, o0 * TILE:(o0 + kt) * TILE])
                    # er expansion: er_strip[:, 4k:4k+4] = OT_k^T @ er_tile
                    ers = ps.tile([TILE, kt * H], f32, tag="ers", bufs=2,
                                  padded_shape=[TILE, Kmax * H], name=f"ers{layer}_{t}")
                    for k in range(kt):
                        nc.tensor.matmul(ers[:, k * H:(k + 1) * H],
                                         lhsT=otsb[:, k, :],
                                         rhs=er_sb[layer][:, t * H:(t + 1) * H],
                                         start=True, stop=True)
                    # batched SDDMM: logits -> lrelu -> exp(s) -> featw
                    lg = wk.tile([TILE, kt * H], f32, tag="lg", bufs=3,
                                 padded_shape=[TILE, Kmax * H], name=f"lg{layer}_{t}")
                    nc.vector.tensor_tensor(lg[:, :], fe[:, :, F:F + H],
                                            ers[:, :], op=ALU.add)
                    lr = wk.tile([TILE, kt * H], f32, tag="lr", bufs=3,
                                 padded_shape=[TILE, Kmax * H], name=f"lr{layer}_{t}")
                    nc.vector.scalar_tensor_tensor(lr[:, :], lg[:, :], NEG, lg[:, :],
                                                   ALU.mult, ALU.max)
                    fw = wk.tile([TILE, kt, F + H], f16, tag="fw", bufs=2,
                                 padded_shape=[TILE, Kmax, F + H], name=f"fw{layer}_{t}")
                    nc.scalar.activation(fw[:, :, F:F + H], lr[:, :], AF.Exp)
                    sv = fw[:, :, F:F + H]
                    s_b = bass.AP(sv.tensor, sv.offset,
                                  [sv.ap[0], [F + H, kt], [1, H], [0, HID]])
                    nc.vector.tensor_tensor(fw[:, :, 0:F], fe[:, :, 0:F], s_b,
                                            op=ALU.mult)
                    agg = ps.tile([TILE, F + H], f32, tag="agg", bufs=2,
                                  name=f"agg{layer}_{t}")
                    for k in range(kt):
                        nc.tensor.matmul(agg[:, :], lhsT=osb[:, k, :],
                                         rhs=fw[:, k, :],
                                         start=(k == 0), stop=(k == kt - 1))
                    yield t, agg

            # warm up the collective stream (absorbs the first-collective
            # barrier while the pre-phase matmuls run)
            wtile = ep.tile([1, ROWE], f16, tag="warm", name="warm")
            nc.vector.memset(wtile[:, :], 0.0)
            nc.sync.dma_start(warm_loc.ap(), wtile[:, :])
            nc.gpsimd.collective_compute(
                "AllGather", mybir.AluOpType.bypass, replica_groups=rg,
                ins=[warm_loc.ap().opt()], outs=[warm_full.ap().opt()])

            # ======= layer-1 pre-phase (+ split AllGathers) =======
            for nt in range(t_tiles):
                pre_tile(xT, Wc1, 0, nt)
                if nt == TA - 1:
                    allgather(0, 0)
            allgather(0, 1)

            # ======= layer 1 edge phase, fused with layer-2 pre-phase =======
            for t, agg in edge_phase(0):
                sl128 = slice(t * TILE, (t + 1) * TILE)
                slF = slice(t * F, (t + 1) * F)
                den = wk.tile([TILE, H], f32, tag="den", name=f"den1_{t}")
                nc.vector.tensor_scalar(den[:, :], agg[:, F:F + H], 1e-9, None, op0=ALU.max)
                rec = wk.tile([TILE, H], f32, tag="rec", name=f"rec1_{t}")
                nc.vector.reciprocal(rec[:, :], den[:, :])
                rst = ep.tile([TILE, F], f32, tag="rst", name=f"rst1_{t}")
                rec_b = bass.AP(rec.tensor, rec.offset,
                                [rec.ap[0], [1, H], [0, HID]])
                nc.vector.tensor_tensor(rst[:, :], agg[:, 0:F], rec_b, op=ALU.mult)
                xb_t = xbp.tile([TILE, F], f32, tag="xb", name=f"xb_{t}")
                nc.sync.dma_start(xb_t[:, :], xb_in[:, slF])
                nc.vector.tensor_tensor(rst[:, :], rst[:, :], xb_t[:, :], op=ALU.add)
                # ELU+1 -> h' = max(rst,0) + exp(min(rst,0))  (the -1 is folded
                # into b2r and the negc2 correction matmul)
                mn = ep.tile([TILE, F], f32, tag="mn", name=f"mn_{t}")
                nc.vector.tensor_scalar(mn[:, :], rst[:, :], 0.0, None, op0=ALU.min)
                r3 = ep.tile([TILE, F], f32, tag="r3", name=f"r3_{t}")
                nc.scalar.activation(r3[:, :], mn[:, :], AF.Exp)
                nc.vector.scalar_tensor_tensor(h_sb[:, slF], rst[:, :], 0.0, r3[:, :],
                                               ALU.max, ALU.add)
                ptr = ps.tile([TILE, TILE], f32, tag="tr", bufs=1, name=f"tr_{t}")
                nc.tensor.transpose(ptr[:, :], h_sb[:, slF], ident[:, :])
                nc.scalar.activation(hT_sb[:, sl128], ptr[:, :], AF.Copy)
                # fused layer-2 pre-phase for this tile
                pre_tile(hT_sb, Wc2, 1, t, corr=True)
                if t == TA - 1:
                    allgather(1, 0)
            allgather(1, 1)

            # ================= layer 2 =================
            for t, agg in edge_phase(1):
                slF = slice(t * F, (t + 1) * F)
                den = wk.tile([TILE, H], f32, tag="den", name=f"den2_{t}")
                nc.vector.tensor_scalar(den[:, :], agg[:, F:F + H], 1e-9, None, op0=ALU.max)
                rec = wk.tile([TILE, H], f32, tag="rec", name=f"rec2_{t}")
                nc.vector.reciprocal(rec[:, :], den[:, :])
                rst = ep.tile([TILE, F], f32, tag="rst", name=f"rst2_{t}")
                rec_b = bass.AP(rec.tensor, rec.offset,
                                [rec.ap[0], [1, H], [0, HID]])
                nc.vector.tensor_tensor(rst[:, :], agg[:, 0:F], rec_b, op=ALU.mult)
                nc.vector.tensor_tensor(rst[:, :], rst[:, :], h_sb[:, slF], op=ALU.add)
                nc.vector.tensor_tensor(rst[:, :], rst[:, :], b2r[:, :], op=ALU.add)
                m1 = ep.tile([TILE, OUTD], f32, tag="m1", name=f"m1_{t}")
                nc.vector.tensor_tensor(m1[:, :], rst[:, 0:OUTD], rst[:, OUTD:2 * OUTD],
                                        op=ALU.add)
                m2 = ep.tile([TILE, OUTD], f32, tag="m2", name=f"m2_{t}")
                nc.vector.tensor_tensor(m2[:, :], rst[:, 2 * OUTD:3 * OUTD],
                                        rst[:, 3 * OUTD:4 * OUTD], op=ALU.add)
                ot = ep.tile([TILE, OUTD], f32, tag="ot", name=f"ot_{t}")
                nc.vector.tensor_tensor(ot[:, :], m1[:, :], m2[:, :], op=ALU.add)
                of = ep.tile([TILE, OUTD], f32, tag="of", name=f"of_{t}")
                nc.vector.tensor_scalar(of[:, :], ot[:, :], 0.25, None, op0=ALU.mult)
                nc.sync.dma_start(out_ext[t * TILE:(t + 1) * TILE, :], of[:, :])

    nc.compile()
    return nc


# ----------------------------------------------------------------------------
# Host entry point
# ----------------------------------------------------------------------------

def make_inputs(x, W1, al1, ar1, b1, W2, al2, ar2, b2, src, dst, n=N, p=P):
    KAs, KBs, per_core = prep_edges(np.asarray(src), np.asarray(dst), n=n, p=p)
    Wcat1 = prep_weights(np.asarray(W1, np.float32), np.asarray(al1, np.float32),
                         np.asarray(ar1, np.float32))
    Wcat2 = prep_weights(np.asarray(W2, np.float32), np.asarray(al2, np.float32),
                         np.asarray(ar2, np.float32))
    node_in = prep_node_inputs(np.asarray(x, np.float32), np.asarray(b1, np.float32),
                               n=n, p=p)
    b2r = np.tile(np.asarray(b2, np.float32)[None, :] - 1.0, (TILE, 1))
    ident = np.eye(TILE, dtype=np.float32)
    ones1 = np.ones((1, TILE), dtype=np.float16)
    negc2 = -Wcat2.astype(np.float32).sum(axis=0, keepdims=True).astype(np.float16)
    in_maps = []
    for c in range(p):
        xT, xb = node_in[c]
        pc = per_core[c]
        gB = pc["gB"] if pc["gB"].shape[1] else np.zeros((TILE, 16), np.int16)
        in_maps.append(dict(
            xT=xT, xb=xb, Wcat1=Wcat1, Wcat2=Wcat2, b2r=b2r, ident=ident,
            ones1=ones1, negc2=negc2,
            gA=pc["gA"], gB=gB, Otab=pc["Otab"], OTtab=pc["OTtab"]))
    return KAs, KBs, in_maps


def kernel(x, W1, al1, ar1, b1, W2, al2, ar2, b2, src, dst, **run_kwargs):
    from concourse.bass_utils import run_bass_kernel_spmd
    KAs, KBs, in_maps = make_inputs(x, W1, al1, ar1, b1, W2, al2, ar2, b2, src, dst)
    nc = build_gat(KAs, KBs)
    res = run_bass_kernel_spmd(nc, in_maps, core_ids=list(range(P)), **run_kwargs)
    out = np.concatenate([r["out"][:NLOC] for r in res.results], axis=0)
    if run_kwargs:
        return out.astype(np.float32), res
    return out.astype(np.float32)
